# revision 4
# baseline (speedup 1.0000x reference)
"""KMeans inference (argmin over squared distances) on 8 Trainium2 cores.

Problem: features [262144, 768] fp32, cluster_centers [1024, 768] fp32.
Output: argmin_k ||x_i - c_k||^2 as int32 [262144].

Strategy (data-parallel over rows):
  - argmin_k ||x-c_k||^2 == argmax_k (x.c_k - 0.5*||c_k||^2); the ||x||^2
    term is constant per row and drops out of the argmin.
  - Shard rows across 8 cores (32768 rows/core). Host pre-transposes each
    shard to xT [768, 32768] so the contraction dim (d) lands on SBUF
    partitions with fully contiguous DMA lines.
  - Per core: scores[m, k] = sum_d xT[d, m] * cT[d, k] via PE matmuls in
    fp32r (full-rate fp32-storage matmul, ~11-bit effective mantissa),
    exact fp32 bias add on GpSimd, argmax over k=1024 with the DVE
    max/max_index instructions.
  - Device also exports each row's top-2 score values. Rows whose top-2
    gap is under a threshold that bounds the fp32r matmul error get an
    exact fp32 recompute on the host (~1% of rows), making the final
    argmin exact at fp32 precision.
"""

import sys

sys.path.insert(0, "/opt/trn_rl_repo")

import numpy as np

N_CORES = 8
N, K, D = 262144, 1024, 768
ROWS_PER_CORE = N // N_CORES          # 32768
SLAB_ROWS = 512                        # rows fetched per DMA slab
N_SLABS = ROWS_PER_CORE // SLAB_ROWS   # 64
SUBTILES = SLAB_ROWS // 128            # 4 row-tiles of 128 per slab
N_ROWTILES = ROWS_PER_CORE // 128      # 256
D_TILES = D // 128                     # 6

# fp32r (~11-bit mantissa) matmul: measured absmax error 8.3e-3 for K=128
# contraction on unit-variance data -> sigma ~1.8e-3, x sqrt(6) for D=768,
# x ~7 sigma tail over 2.7e8 samples -> |score error| < ~3e-2, top-2 gap
# error < ~6e-2.  GAP_THRESHOLD = 0.25 leaves >4x margin.
GAP_THRESHOLD = 0.25

_PROGRAM = None


def _build_program():
    import concourse.mybir as mybir
    from concourse import bacc
    from concourse.tile import TileContext

    F32 = mybir.dt.float32
    F32R = mybir.dt.float32r
    U32 = mybir.dt.uint32

    nc = bacc.Bacc()
    # Inputs (per core): transposed feature shard, transposed centroids,
    # bias tile (-0.5*||c_k||^2 replicated over 128 partitions).
    xt = nc.declare_dram_parameter("xt", [D, ROWS_PER_CORE], F32R, isOutput=False)
    cbt = nc.declare_dram_parameter("cbt", [D, K], F32R, isOutput=False)
    bias = nc.declare_dram_parameter("bias", [128, K], F32, isOutput=False)
    # Outputs: idx[p, m] = argmax index of row m*128 + p; top2[p, 2m:2m+2]
    # = top-2 score values of that row.
    out_idx = nc.declare_dram_parameter("idx", [128, N_ROWTILES], U32, isOutput=True)
    out_top2 = nc.declare_dram_parameter(
        "top2", [128, 2 * N_ROWTILES], F32, isOutput=True
    )

    with TileContext(nc) as tc:
        with (
            tc.tile_pool(name="consts", bufs=1) as consts,
            tc.tile_pool(name="xslab", bufs=3) as xslab_pool,
            tc.tile_pool(name="scores", bufs=3) as scores_pool,
            tc.tile_pool(name="maxes", bufs=6) as maxes_pool,
            tc.tile_pool(name="psum", bufs=8, space="PSUM") as psum_pool,
        ):
            # Centroids resident in SBUF: 6 tiles [128, 1024] + bias tile.
            cb = consts.tile([128, D_TILES, K], F32R, tag="cb")
            nc.sync.dma_start(
                out=cb,
                in_=cbt.rearrange("(t p) k -> p t k", p=128),
            )
            bias_t = consts.tile([128, K], F32, tag="bias")
            nc.sync.dma_start(out=bias_t, in_=bias[:, :])

            staging_idx = consts.tile([128, N_ROWTILES], U32, tag="sidx")
            staging_top2 = consts.tile([128, 2 * N_ROWTILES], F32, tag="stop2")

            for slab in range(N_SLABS):
                r0 = slab * SLAB_ROWS
                xs = xslab_pool.tile([128, D_TILES, SLAB_ROWS], F32R, tag="xs")
                nc.sync.dma_start(
                    out=xs,
                    in_=xt.rearrange("(t p) r -> p t r", p=128)[
                        :, :, r0 : r0 + SLAB_ROWS
                    ],
                )
                for sub in range(SUBTILES):
                    m = slab * SUBTILES + sub
                    scores = scores_pool.tile([128, K], F32, tag="scores")
                    for half in range(2):
                        k0 = half * 512
                        ps = psum_pool.tile([128, 512], F32, tag="ps")
                        for dt in range(D_TILES):
                            nc.tensor.matmul(
                                ps,
                                xs[:, dt, sub * 128 : (sub + 1) * 128],
                                cb[:, dt, k0 : k0 + 512],
                                start=(dt == 0),
                                stop=(dt == D_TILES - 1),
                            )
                        nc.scalar.copy(scores[:, k0 : k0 + 512], ps)
                    # exact fp32 bias add on the (otherwise idle) GpSimd
                    nc.gpsimd.tensor_add(scores, scores, bias_t)
                    max8 = maxes_pool.tile([128, 8], F32, tag="max8")
                    idx8 = maxes_pool.tile([128, 8], U32, tag="idx8")
                    nc.vector.max(out=max8, in_=scores)
                    nc.vector.max_index(out=idx8, in_max=max8, in_values=scores)
                    nc.scalar.copy(staging_idx[:, m : m + 1], idx8[:, 0:1])
                    nc.scalar.copy(
                        staging_top2[:, 2 * m : 2 * m + 2], max8[:, 0:2]
                    )

            nc.sync.dma_start(out=out_idx[:, :], in_=staging_idx)
            nc.sync.dma_start(out=out_top2[:, :], in_=staging_top2)

    nc.finalize()
    return nc


def _get_program():
    global _PROGRAM
    if _PROGRAM is None:
        _PROGRAM = _build_program()
    return _PROGRAM


def _make_in_maps(features, cluster_centers):
    cbt = np.ascontiguousarray(cluster_centers.T)  # [768, 1024]
    c2 = (cluster_centers.astype(np.float64) ** 2).sum(axis=1)
    bias_row = (-0.5 * c2).astype(np.float32)
    bias = np.ascontiguousarray(np.broadcast_to(bias_row, (128, K)))

    in_maps = []
    for i in range(N_CORES):
        shard = features[i * ROWS_PER_CORE : (i + 1) * ROWS_PER_CORE]
        xtr = np.ascontiguousarray(shard.T)  # [768, 32768]
        in_maps.append({"xt": xtr, "cbt": cbt, "bias": bias})
    return in_maps


def _postprocess(res, features, cluster_centers):
    """Assemble indices; exactly recompute rows with a small top-2 gap."""
    idx_parts = []
    gap_parts = []
    for i in range(N_CORES):
        idx = res.results[i]["idx"]          # [128, 256] uint32
        top2 = res.results[i]["top2"]        # [128, 512] fp32
        idx_parts.append(idx.T.reshape(-1))  # row r = m*128 + p
        t2 = top2.reshape(128, N_ROWTILES, 2).transpose(1, 0, 2).reshape(-1, 2)
        gap_parts.append(t2[:, 0] - t2[:, 1])
    out = np.concatenate(idx_parts).astype(np.int32)
    gap = np.concatenate(gap_parts)

    risky = np.flatnonzero(gap < GAP_THRESHOLD)
    if risky.size:
        x = features[risky]
        s = x @ cluster_centers.T
        s += -0.5 * (cluster_centers * cluster_centers).sum(axis=1)
        out[risky] = s.argmax(axis=1).astype(np.int32)
    return out


def kernel(features: np.ndarray, cluster_centers: np.ndarray) -> np.ndarray:
    from concourse.bass_utils import run_bass_kernel_spmd

    features = np.ascontiguousarray(features, dtype=np.float32)
    cluster_centers = np.ascontiguousarray(cluster_centers, dtype=np.float32)

    in_maps = _make_in_maps(features, cluster_centers)
    nc = _get_program()
    res = run_bass_kernel_spmd(nc, in_maps, core_ids=list(range(N_CORES)))
    return _postprocess(res, features, cluster_centers)


if __name__ == "__main__":
    rng = np.random.default_rng(0)
    f = rng.standard_normal((N, D)).astype(np.float32)
    c = rng.standard_normal((K, D)).astype(np.float32)
    got = kernel(f, c)
    d2 = (
        (f**2).sum(1, keepdims=True)
        - 2.0 * f @ c.T
        + (c**2).sum(1)
    )
    want = d2.argmin(1)
    print("mismatches:", (got != want).sum(), "/", N)
